# revision 21
# baseline (speedup 1.0000x reference)
"""MoE (top-2 of 8 experts, SwiGLU) on 8 Trainium2 NeuronCores.

Strategy (expert-parallel + quarter-expert load balancing):
  - Host computes the router and the top-2 dispatch (exact fp32 replica of
    the reference), yielding per-expert token lists + combine weights.
  - Each expert's MLP is split into TWO half-inter jobs (inter rows
    [h*1024,(h+1)*1024)): a job runs GEMM1 for its half of the gate/up
    rows and GEMM2 contracted over its half of INTER, producing a partial
    output for all of its expert's tokens. The halves are exact partial
    sums, added on the host. (A quarter-split S=4 was tried and rejected:
    its 4-deep GEMM2 accumulation groups are too short to hide the
    combine-mult latency in the 2-buffer PSUM ring, costing more than the
    extra balance saves.)
  - The 16 jobs are packed 2-per-core: slot s takes the jobs ranked
    [8s, 8s+8) by token count, padded to the slot max (538+508 columns for
    the key(0) routing vs 2*538 unbalanced) — the PE-bound cost scales
    with padded columns.
  - Per job: hT = w1q[j] @ x_jT (gate/up fused), yT = silu(g)*u,
    oT = (w2q[j] @ yT) * combine. All GEMMs bf16 with fp32 PSUM; outputs
    ship bf16 (host adds partials in fp32). fp8 was evaluated and rejected:
    plain e4m3 fails the 2e-2 gate (6.5e-2), and hi+lo residual
    compensation needs 3x the products, which loses at the ~1.9x measured
    DoubleRow speedup.
  - Emission: all GEMM1s first (job j+1's GEMM1 hides job j's yt latency,
    and w2 DMAs queue after all w1), then GEMM2s with the job owning the
    smallest final chunk last — the exposed tail (combine-mult + out DMA +
    drain) scales with the final chunk width.

Layouts keep tokens on the PSUM free dim everywhere so no on-device
transposes are needed; weights are pre-transposed on the host.
"""

import sys

sys.path.insert(0, "/opt/trn_rl_repo")

import numpy as np
import ml_dtypes

import concourse.bass as bass  # noqa: F401  (bass must import before tile)
import concourse.tile as tile
from concourse import bacc, mybir
from concourse.bass_utils import run_bass_kernel_spmd

T = 2048
H = 1024
INTER = 2048
S = 2                  # inter-dim splits per expert (jobs per core)
IH = INTER // S        # 1024 inter rows per job
E = 8
TOPK = 2
N_CORES = 8
P = 128

DT = mybir.dt.bfloat16
NP_DT = ml_dtypes.bfloat16

_PROGRAM_CACHE = {}    # cs tuple -> compiled Bacc program

KH = H // P            # 8  k-tiles for GEMM1 (contract over H)
KI = IH // P           # 8  k-tiles for GEMM2 (contract over half INTER)
NPAIR = IH // P        # 8  gate/up pairs per job
NH = H // P            # 8  output h-tiles


def _route(x, router_w):
    """Replicates the reference router in fp32 numpy.

    Returns per-expert (token_indices, combine_weights)."""
    gating = (x @ router_w.T).astype(np.float32)              # [T, E]
    m = gating.max(axis=1, keepdims=True)
    p = np.exp(gating - m, dtype=np.float32)
    probs = p / p.sum(axis=1, keepdims=True)
    order = np.argsort(-probs, axis=1, kind="stable")         # ties -> lower idx
    sel = order[:, :TOPK]                                     # [T, K]
    topw = np.take_along_axis(probs, sel, axis=1)             # [T, K]

    idxs, wts = [], []
    for e in range(E):
        m_e = sel == e                                        # [T, K]
        rows = np.nonzero(m_e.any(axis=1))[0]
        idxs.append(rows.astype(np.int64))
        wts.append(topw[m_e].astype(np.float32))              # aligned with rows
    return idxs, wts


def _assign_jobs(loads):
    """S*E quarter-expert jobs -> N_CORES cores x S slots.

    Slot s holds the 8 jobs ranked [8s, 8s+8) by token count, padded to the
    slot max. Returns (jobs_per_core, cs): jobs_per_core[core][s] = (e, q),
    cs[s] = padded column count of slot s."""
    jobs = sorted(((loads[e], e, q) for e in range(E) for q in range(S)),
                  reverse=True)
    jobs_per_core = [[None] * S for _ in range(N_CORES)]
    cs = []
    for s in range(S):
        block = jobs[s * N_CORES:(s + 1) * N_CORES]
        cs.append(max(64, -(-block[0][0] // 2) * 2))
        for core, (_, e, q) in enumerate(block):
            jobs_per_core[core][s] = (e, q)
    return jobs_per_core, tuple(cs)


def _chunks(c):
    """Split c tokens into near-equal chunks of <=512 (PSUM bank limit).

    Chunks are kept >=256 where possible: below that LDWEIGHTS (~107 ns)
    stops hiding under the matmul stream and the PE goes weight-load-bound."""
    n = -(-c // 512)
    base = -(-(-(-c // n)) // 4) * 4                          # ceil(c/n) to mult of 4
    sizes = []
    left = c
    for _ in range(n - 1):
        sizes.append(base)
        left -= base
    sizes.append(left)
    return [s for s in sizes if s > 0]


def _build_program(cs, loop_n=0):
    """One SPMD program: S quarter-expert jobs (cs[s] padded tokens each).

    loop_n > 0 wraps the body in an on-device For_i loop (used only by the
    perf harness to measure the per-iteration slope)."""
    nc = bacc.Bacc("TRN2", target_bir_lowering=False, debug=False,
                   num_devices=N_CORES)
    f32 = mybir.dt.float32
    xt_d, w1t_d, w2t_d, sc_d, out_d = {}, {}, {}, {}, {}
    for j in range(S):
        xt_d[j] = nc.dram_tensor(f"xt{j}", [H, cs[j]], DT,
                                 kind="ExternalInput").ap()
        w1t_d[j] = nc.dram_tensor(f"w1t{j}", [H, 2 * IH], DT,
                                  kind="ExternalInput").ap()
        w2t_d[j] = nc.dram_tensor(f"w2t{j}", [IH, H], DT,
                                  kind="ExternalInput").ap()
        sc_d[j] = nc.dram_tensor(f"scale{j}", [P, cs[j]], f32,
                                 kind="ExternalInput").ap()
        out_d[j] = nc.dram_tensor(f"out{j}", [H, cs[j]], DT,
                                  kind="ExternalOutput").ap()

    from contextlib import ExitStack
    with tile.TileContext(nc) as tc, ExitStack() as ctx:
        wpool = ctx.enter_context(tc.tile_pool(name="weights", bufs=1))
        xpool = ctx.enter_context(tc.tile_pool(name="xt", bufs=1))
        ypool = ctx.enter_context(tc.tile_pool(name="yt", bufs=1))
        apool = ctx.enter_context(tc.tile_pool(name="act", bufs=2))
        opool = ctx.enter_context(tc.tile_pool(name="ot", bufs=1))
        pgpool = ctx.enter_context(tc.tile_pool(name="psg", bufs=3, space="PSUM"))
        pupool = ctx.enter_context(tc.tile_pool(name="psu", bufs=3, space="PSUM"))
        popool = ctx.enter_context(tc.tile_pool(name="pso", bufs=2, space="PSUM"))

        if loop_n:
            loop = ctx.enter_context(tc.For_i(
                0, loop_n, 1,
                hint_engines=(mybir.EngineType.PE, mybir.EngineType.SP,
                              mybir.EngineType.Activation, mybir.EngineType.DVE)))

        # ---- PE warmup ----
        # Dependency-free matmuls on an (uninitialized) scratch tile warm the
        # PE HAM clock-gate to 2.4 GHz during the initial DMA wait.
        warm_sb = xpool.tile([P, P], DT, tag="warm")
        nc.vector.memset(warm_sb[:, 0:1], 0.0)
        ps_w = popool.tile([P, P], f32, tag="pso", name="ps_warm")
        for _ in range(54):
            nc.tensor.matmul(ps_w[:], lhsT=warm_sb[:], rhs=warm_sb[:],
                             start=True, stop=True)

        # ---- input loads ----
        # One merged DMA per logical tensor/piece: HWDGE prep (~625 ns) is
        # per-instruction and serialized with transfers on the single queue,
        # so fewer+bigger DMAs win; the leading pieces gate the first MMs.
        xt_t, xt_sb, sc_sb, w1p = {}, {}, {}, {}
        chunk_sizes = {j: _chunks(cs[j]) for j in range(S)}

        W1PC = 512  # w1 piece: 512 cols (4 pairs' gate or up halves)

        def load_w1_cols(j, lo, hi, tag):
            t = wpool.tile([P, KH, hi - lo], DT, tag=tag, name=tag)
            nc.sync.dma_start(
                out=t[:], in_=w1t_d[j][:, lo:hi].rearrange("(k p) c -> p k c", p=P))
            return t

        # job 0 leads: x chunk-1, first 256 gate cols, rest of gate, x rest,
        # then the up piece — in PE consumption order.
        xt_t[0] = xpool.tile([P, KH, cs[0]], DT, tag="xt0", name="xt0")
        xv0 = xt_d[0].rearrange("(k p) c -> p k c", p=P)
        c1 = chunk_sizes[0][0]
        nc.sync.dma_start(out=xt_t[0][:, :, :c1], in_=xv0[:, :, :c1])
        w1_0a = wpool.tile([P, KH, 2 * P], DT, tag="w1_0a", name="w1_0a")
        w1_0a_view = w1t_d[0][:, :2 * P].rearrange("(k p) c -> p k c", p=P)
        nc.sync.dma_start(out=w1_0a[:], in_=w1_0a_view[:])
        w1p[(0, "0b")] = load_w1_cols(0, 2 * P, W1PC, "w1_0_0b")
        if c1 < cs[0]:
            nc.sync.dma_start(out=xt_t[0][:, :, c1:], in_=xv0[:, :, c1:])
        # remaining job-0 pieces: gate piece p feeds pairs 4p..4p+3 paired
        # with up piece p+2
        for piece in (2, 1, 3):
            w1p[(0, piece)] = load_w1_cols(0, piece * W1PC, (piece + 1) * W1PC,
                                           f"w1_0_{piece}")
        # jobs 1..S-1: x then w1 pieces in consumption order
        for j in range(1, S):
            xt_t[j] = xpool.tile([P, KH, cs[j]], DT, tag=f"xt{j}",
                                 name=f"xt{j}")
            nc.sync.dma_start(out=xt_t[j][:],
                              in_=xt_d[j].rearrange("(k p) c -> p k c", p=P))
            for piece in (0, 2, 1, 3):
                w1p[(j, piece)] = load_w1_cols(j, piece * W1PC,
                                               (piece + 1) * W1PC,
                                               f"w1_{j}_{piece}")

        for j in range(S):
            xt_sb[j] = [xt_t[j][:, k, :] for k in range(KH)]

        # w2: one merged DMA per job, after all w1
        w2_sb = {}
        for j in range(S):
            t = wpool.tile([P, KI, H], DT, tag=f"w2_{j}", name=f"w2_{j}")
            nc.sync.dma_start(
                out=t[:], in_=w2t_d[j].rearrange("(k p) c -> p k c", p=P))
            w2_sb[j] = [t[:, k, :] for k in range(KI)]

        for j in range(S):
            sc_sb[j] = xpool.tile([P, cs[j]], f32, tag=f"sc{j}", name=f"sc{j}")
            nc.sync.dma_start(out=sc_sb[j][:], in_=sc_d[j][:])

        def w1_slice(j, k, i):
            # stationary lhsT [P(h), P(inter)] for job-local inter tile i
            # (0..2*NPAIR-1: NPAIR gate then NPAIR up)
            piece, sub = divmod(i, W1PC // P)
            if j == 0 and piece == 0:
                if sub < 2:
                    return w1_0a[:, k, P * sub:P * (sub + 1)]
                return w1p[(0, "0b")][:, k, P * (sub - 2):P * (sub - 1)]
            return w1p[(j, piece)][:, k, P * sub:P * (sub + 1)]

        csls = {}
        for j in range(S):
            csls[j] = []
            c0 = 0
            for cn in chunk_sizes[j]:
                csls[j].append((slice(c0, c0 + cn), cn))
                c0 += cn

        yt_sb = {}

        def gemm1(j):
            # yT[i] = silu(gate_i) * up_i, [P, c] per pair i. Quad structure:
            # 4 gate pairs then their 4 ups so the PE has gate work while the
            # up piece streams.
            yt_sb[j] = [None] * NPAIR
            for q in range(NPAIR // 4):
                quad = range(4 * q, 4 * q + 4)
                sgs = {}
                for i in quad:
                    yt_sb[j][i] = ypool.tile([P, cs[j]], DT, tag=f"yt{j}_{i}",
                                             name=f"yt{j}_{i}")
                for ci, (csl, cn) in enumerate(csls[j]):
                    for i in quad:
                        ps_g = pgpool.tile([P, cn], f32, tag="psg")
                        for k in range(KH):
                            nc.tensor.matmul(ps_g[:], lhsT=w1_slice(j, k, i),
                                             rhs=xt_sb[j][k][:, csl],
                                             start=(k == 0), stop=(k == KH - 1))
                        sg = apool.tile([P, cn], f32, tag=f"sg{i % 4}_{ci}")
                        nc.scalar.activation(sg[:], ps_g[:],
                                             mybir.ActivationFunctionType.Silu)
                        sgs[(i, ci)] = sg
                for ci, (csl, cn) in enumerate(csls[j]):
                    for i in quad:
                        ps_u = pupool.tile([P, cn], f32, tag="psu")
                        for k in range(KH):
                            nc.tensor.matmul(ps_u[:],
                                             lhsT=w1_slice(j, k, i + NPAIR),
                                             rhs=xt_sb[j][k][:, csl],
                                             start=(k == 0), stop=(k == KH - 1))
                        nc.vector.tensor_mul(yt_sb[j][i][:, csl],
                                             sgs[(i, ci)][:], ps_u[:])

        def gemm2(j, last=False):
            # chunk-outer / h-inner so each chunk's 8 h-tiles land in one
            # [P, NH, cn] slice of o_big and ship as ONE DMA — few out DMAs
            # keep the serialized HWDGE prep off the critical path. The very
            # last chunk instead ships per-h so the final transfers overlap
            # the remaining h-groups.
            o_big = opool.tile([P, NH, cs[j]], DT, tag=f"o{j}", name=f"o{j}")
            ov = out_d[j].rearrange("(h p) c -> p h c", p=P)
            n_chunks = len(csls[j])
            for ci, (csl, cn) in enumerate(csls[j]):
                tail = last and ci == n_chunks - 1
                for jh in range(NH):
                    ps_o = popool.tile([P, cn], f32, tag="pso")
                    for k in range(KI):
                        nc.tensor.matmul(
                            ps_o[:], lhsT=w2_sb[j][k][:, P * jh:P * (jh + 1)],
                            rhs=yt_sb[j][k][:, csl],
                            start=(k == 0), stop=(k == KI - 1))
                    if tail:
                        o_h = opool.tile([P, 1, cn], DT, tag=f"oh{jh}",
                                         name=f"oh{j}_{jh}")
                        nc.vector.tensor_mul(o_h[:, 0, :],
                                             sc_sb[j][:, csl], ps_o[:])
                        nc.sync.dma_start(out=ov[:, jh:jh + 1, csl],
                                          in_=o_h[:])
                    else:
                        nc.vector.tensor_mul(o_big[:, jh, csl],
                                             sc_sb[j][:, csl], ps_o[:])
                if not tail:
                    nc.sync.dma_start(out=ov[:, :, csl], in_=o_big[:, :, csl])

        for j in range(S):
            gemm1(j)
        # emit last the job whose final chunk is smallest: the tail
        # (mult + out DMA + drain) scales with the final chunk width
        order = sorted(range(S), key=lambda j: chunk_sizes[j][-1],
                       reverse=True)
        for j in order[:-1]:
            gemm2(j)
        gemm2(order[-1], last=True)

    nc.compile()
    return nc


def _make_in_maps(x, w1, w2, router_w):
    """Route + build per-core input shards. Returns (in_maps, meta) where
    meta = (idxs, jobs_per_core, cs) for unsharding."""
    idxs, wts = _route(x, router_w)
    loads = [len(i) for i in idxs]
    jobs_per_core, cs = _assign_jobs(loads)

    xt_f32 = x.T  # [H, T]
    cache = {}

    def job_tensors(e, q, c_pad):
        n = len(idxs[e])
        if (e, c_pad) not in cache:
            xt = np.zeros((H, c_pad), dtype=NP_DT)
            xt[:, :n] = xt_f32[:, idxs[e]].astype(NP_DT)
            sc = np.zeros((P, c_pad), dtype=np.float32)
            sc[:, :n] = wts[e][None, :]
            cache[(e, c_pad)] = (xt, sc)
        xt, sc = cache[(e, c_pad)]
        # gate rows [q*IH,(q+1)*IH) and up rows [INTER+q*IH, INTER+(q+1)*IH)
        w1j = np.concatenate([w1[e][q * IH:(q + 1) * IH],
                              w1[e][INTER + q * IH:INTER + (q + 1) * IH]],
                             axis=0)
        w2j = w2[e][:, q * IH:(q + 1) * IH]
        return {
            "xt": xt,
            "w1t": np.ascontiguousarray(w1j.T).astype(NP_DT),
            "w2t": np.ascontiguousarray(w2j.T).astype(NP_DT),
            "scale": sc,
        }

    in_maps = []
    for core in range(N_CORES):
        m = {}
        for s in range(S):
            e, q = jobs_per_core[core][s]
            t = job_tensors(e, q, cs[s])
            m[f"xt{s}"] = t["xt"]
            m[f"w1t{s}"] = t["w1t"]
            m[f"w2t{s}"] = t["w2t"]
            m[f"scale{s}"] = t["scale"]
        in_maps.append(m)
    return in_maps, (idxs, jobs_per_core, cs)


def kernel(hidden_states, w1, w2, router_w):
    x = np.ascontiguousarray(np.asarray(hidden_states, dtype=np.float32)
                             .reshape(T, H))
    w1 = np.asarray(w1, dtype=np.float32)
    w2 = np.asarray(w2, dtype=np.float32)
    router_w = np.asarray(router_w, dtype=np.float32)

    in_maps, (idxs, jobs_per_core, cs) = _make_in_maps(x, w1, w2, router_w)

    nc = _PROGRAM_CACHE.get(cs)
    if nc is None:
        nc = _PROGRAM_CACHE[cs] = _build_program(cs)

    try:
        res = run_bass_kernel_spmd(nc, in_maps, list(range(N_CORES)))
    except Exception:
        # transient runtime hiccups (e.g. mesh desync on a fresh session)
        # usually clear on retry
        res = run_bass_kernel_spmd(nc, in_maps, list(range(N_CORES)))

    out = np.zeros((T, H), dtype=np.float32)
    for core in range(N_CORES):
        for s in range(S):
            e, _q = jobs_per_core[core][s]
            n = len(idxs[e])
            if n:
                out[idxs[e]] += res.results[core][f"out{s}"][:, :n].T
    return out.reshape(1, T, H)


# revision 27
# speedup vs baseline: 1.0544x; 1.0544x over previous
"""MoE (top-2 of 8 experts, SwiGLU) on 8 Trainium2 NeuronCores.

Strategy (expert-parallel + quarter-expert load balancing):
  - Host computes the router and the top-2 dispatch (exact fp32 replica of
    the reference), yielding per-expert token lists + combine weights.
  - Each expert's MLP is split into TWO half-inter jobs (inter rows
    [h*1024,(h+1)*1024)): a job runs GEMM1 for its half of the gate/up
    rows and GEMM2 contracted over its half of INTER, producing a partial
    output for all of its expert's tokens. The halves are exact partial
    sums, added on the host. (A quarter-split S=4 was tried and rejected:
    its 4-deep GEMM2 accumulation groups are too short to hide the
    combine-mult latency in the 2-buffer PSUM ring, costing more than the
    extra balance saves.)
  - The 16 jobs are packed 2-per-core: slot s takes the jobs ranked
    [8s, 8s+8) by token count, padded to the slot max (538+508 columns for
    the key(0) routing vs 2*538 unbalanced) — the PE-bound cost scales
    with padded columns.
  - Per job: hT = w1q[j] @ x_jT (gate/up fused), yT = silu(g)*u,
    oT = (w2q[j] @ yT) * combine. All GEMMs bf16 with fp32 PSUM; outputs
    ship bf16 (host adds partials in fp32). fp8 was evaluated and rejected:
    plain e4m3 fails the 2e-2 gate (6.5e-2), and hi+lo residual
    compensation needs 3x the products, which loses at the ~1.9x measured
    DoubleRow speedup.
  - Emission: all GEMM1s first (job j+1's GEMM1 hides job j's yt latency,
    and w2 DMAs queue after all w1), then GEMM2s with the job owning the
    smallest final chunk last — the exposed tail (combine-mult + out DMA +
    drain) scales with the final chunk width.

Layouts keep tokens on the PSUM free dim everywhere so no on-device
transposes are needed; weights are pre-transposed on the host.
"""

import sys

sys.path.insert(0, "/opt/trn_rl_repo")

import numpy as np
import ml_dtypes

import concourse.bass as bass  # noqa: F401  (bass must import before tile)
import concourse.tile as tile
from concourse import bacc, mybir
from concourse.bass_utils import run_bass_kernel_spmd

T = 2048
H = 1024
INTER = 2048
S = 2                  # inter-dim splits per expert (jobs per core)
IH = INTER // S        # 1024 inter rows per job
E = 8
TOPK = 2
N_CORES = 8
P = 128

DT = mybir.dt.bfloat16
NP_DT = ml_dtypes.bfloat16

_PROGRAM_CACHE = {}    # cs tuple -> compiled Bacc program

KH = H // P            # 8  k-tiles for GEMM1 (contract over H)
KI = IH // P           # 8  k-tiles for GEMM2 (contract over half INTER)
NPAIR = IH // P        # 8  gate/up pairs per job
NH = H // P            # 8  output h-tiles


def _route(x, router_w):
    """Replicates the reference router in fp32 numpy.

    Returns per-expert (token_indices, combine_weights)."""
    gating = (x @ router_w.T).astype(np.float32)              # [T, E]
    m = gating.max(axis=1, keepdims=True)
    p = np.exp(gating - m, dtype=np.float32)
    probs = p / p.sum(axis=1, keepdims=True)
    order = np.argsort(-probs, axis=1, kind="stable")         # ties -> lower idx
    sel = order[:, :TOPK]                                     # [T, K]
    topw = np.take_along_axis(probs, sel, axis=1)             # [T, K]

    idxs, wts = [], []
    for e in range(E):
        m_e = sel == e                                        # [T, K]
        rows = np.nonzero(m_e.any(axis=1))[0]
        idxs.append(rows.astype(np.int64))
        wts.append(topw[m_e].astype(np.float32))              # aligned with rows
    return idxs, wts


def _assign_jobs(loads):
    """S*E quarter-expert jobs -> N_CORES cores x S slots.

    Slot s holds the 8 jobs ranked [8s, 8s+8) by token count, padded to the
    slot max. Returns (jobs_per_core, cs): jobs_per_core[core][s] = (e, q),
    cs[s] = padded column count of slot s."""
    jobs = sorted(((loads[e], e, q) for e in range(E) for q in range(S)),
                  reverse=True)
    jobs_per_core = [[None] * S for _ in range(N_CORES)]
    cs = []
    for s in range(S):
        block = jobs[s * N_CORES:(s + 1) * N_CORES]
        cs.append(max(64, -(-block[0][0] // 2) * 2))
        for core, (_, e, q) in enumerate(block):
            jobs_per_core[core][s] = (e, q)
    return jobs_per_core, tuple(cs)


def _chunks(c):
    """Split c tokens into near-equal chunks of <=512 (PSUM bank limit).

    Chunks are kept >=256 where possible: below that LDWEIGHTS (~107 ns)
    stops hiding under the matmul stream and the PE goes weight-load-bound."""
    n = -(-c // 512)
    base = -(-(-(-c // n)) // 4) * 4                          # ceil(c/n) to mult of 4
    sizes = []
    left = c
    for _ in range(n - 1):
        sizes.append(base)
        left -= base
    sizes.append(left)
    return [s for s in sizes if s > 0]


def _build_program(cs, loop_n=0):
    """One SPMD program: S quarter-expert jobs (cs[s] padded tokens each).

    loop_n > 0 wraps the body in an on-device For_i loop (used only by the
    perf harness to measure the per-iteration slope)."""
    nc = bacc.Bacc("TRN2", target_bir_lowering=False, debug=False,
                   num_devices=N_CORES)
    f32 = mybir.dt.float32
    xt_d, w1t_d, w2t_d, sc_d, out_d = {}, {}, {}, {}, {}
    for j in range(S):
        xt_d[j] = nc.dram_tensor(f"xt{j}", [H, cs[j]], DT,
                                 kind="ExternalInput").ap()
        w1t_d[j] = nc.dram_tensor(f"w1t{j}", [H, 2 * IH], DT,
                                  kind="ExternalInput").ap()
        w2t_d[j] = nc.dram_tensor(f"w2t{j}", [IH, H], DT,
                                  kind="ExternalInput").ap()
        sc_d[j] = nc.dram_tensor(f"scale{j}", [P, cs[j]], f32,
                                 kind="ExternalInput").ap()
        out_d[j] = nc.dram_tensor(f"out{j}", [H, cs[j]], DT,
                                  kind="ExternalOutput").ap()

    from contextlib import ExitStack
    with tile.TileContext(nc) as tc, ExitStack() as ctx:
        wpool = ctx.enter_context(tc.tile_pool(name="weights", bufs=1))
        xpool = ctx.enter_context(tc.tile_pool(name="xt", bufs=1))
        ypool = ctx.enter_context(tc.tile_pool(name="yt", bufs=1))
        apool = ctx.enter_context(tc.tile_pool(name="act", bufs=2))
        opool = ctx.enter_context(tc.tile_pool(name="ot", bufs=1))
        pgpool = ctx.enter_context(tc.tile_pool(name="psg", bufs=3, space="PSUM"))
        pupool = ctx.enter_context(tc.tile_pool(name="psu", bufs=3, space="PSUM"))
        popool = ctx.enter_context(tc.tile_pool(name="pso", bufs=2, space="PSUM"))

        if loop_n:
            loop = ctx.enter_context(tc.For_i(
                0, loop_n, 1,
                hint_engines=(mybir.EngineType.PE, mybir.EngineType.SP,
                              mybir.EngineType.Activation, mybir.EngineType.DVE)))

        # ---- PE warmup ----
        # Dependency-free matmuls on an (uninitialized) scratch tile warm the
        # PE HAM clock-gate to 2.4 GHz during the initial DMA wait.
        warm_sb = xpool.tile([P, P], DT, tag="warm")
        nc.vector.memset(warm_sb[:, 0:1], 0.0)
        ps_w = popool.tile([P, P], f32, tag="pso", name="ps_warm")
        for _ in range(44):
            nc.tensor.matmul(ps_w[:], lhsT=warm_sb[:], rhs=warm_sb[:],
                             start=True, stop=True)

        # ---- input loads ----
        # One merged DMA per logical tensor/piece: HWDGE prep (~625 ns) is
        # per-instruction and serialized with transfers on the single queue,
        # so fewer+bigger DMAs win; the leading pieces gate the first MMs.
        xt_t, xt_sb, sc_sb, w1p = {}, {}, {}, {}
        chunk_sizes = {j: _chunks(cs[j]) for j in range(S)}

        W1PC = 512  # w1 piece: 512 cols (4 pairs' gate or up halves)

        def load_w1_cols(j, lo, hi, tag):
            t = wpool.tile([P, KH, hi - lo], DT, tag=tag, name=tag)
            nc.sync.dma_start(
                out=t[:], in_=w1t_d[j][:, lo:hi].rearrange("(k p) c -> p k c", p=P))
            return t

        # job 0 leads: x chunk-1, first 256 gate cols, rest of gate, x rest,
        # then the up piece — in PE consumption order.
        xt_t[0] = xpool.tile([P, KH, cs[0]], DT, tag="xt0", name="xt0")
        xv0 = xt_d[0].rearrange("(k p) c -> p k c", p=P)
        c1 = chunk_sizes[0][0]
        nc.sync.dma_start(out=xt_t[0][:, :, :c1], in_=xv0[:, :, :c1])
        w1_0a = wpool.tile([P, KH, 2 * P], DT, tag="w1_0a", name="w1_0a")
        w1_0a_view = w1t_d[0][:, :2 * P].rearrange("(k p) c -> p k c", p=P)
        nc.sync.dma_start(out=w1_0a[:], in_=w1_0a_view[:])
        w1p[(0, "0b")] = load_w1_cols(0, 2 * P, W1PC, "w1_0_0b")
        if c1 < cs[0]:
            nc.sync.dma_start(out=xt_t[0][:, :, c1:], in_=xv0[:, :, c1:])
        # remaining job-0 pieces: gate piece p feeds pairs 4p..4p+3 paired
        # with up piece p+2
        for piece in (2, 1, 3):
            w1p[(0, piece)] = load_w1_cols(0, piece * W1PC, (piece + 1) * W1PC,
                                           f"w1_0_{piece}")
        # jobs 1..S-1: x then w1 pieces in consumption order
        for j in range(1, S):
            xt_t[j] = xpool.tile([P, KH, cs[j]], DT, tag=f"xt{j}",
                                 name=f"xt{j}")
            nc.sync.dma_start(out=xt_t[j][:],
                              in_=xt_d[j].rearrange("(k p) c -> p k c", p=P))
            for piece in (0, 2, 1, 3):
                w1p[(j, piece)] = load_w1_cols(j, piece * W1PC,
                                               (piece + 1) * W1PC,
                                               f"w1_{j}_{piece}")

        for j in range(S):
            xt_sb[j] = [xt_t[j][:, k, :] for k in range(KH)]

        # w2: one merged DMA per job, after all w1
        w2_sb = {}
        for j in range(S):
            t = wpool.tile([P, KI, H], DT, tag=f"w2_{j}", name=f"w2_{j}")
            nc.sync.dma_start(
                out=t[:], in_=w2t_d[j].rearrange("(k p) c -> p k c", p=P))
            w2_sb[j] = [t[:, k, :] for k in range(KI)]

        for j in range(S):
            sc_sb[j] = xpool.tile([P, cs[j]], f32, tag=f"sc{j}", name=f"sc{j}")
            nc.sync.dma_start(out=sc_sb[j][:], in_=sc_d[j][:])

        def w1_slice(j, k, i):
            # stationary lhsT [P(h), P(inter)] for job-local inter tile i
            # (0..2*NPAIR-1: NPAIR gate then NPAIR up)
            piece, sub = divmod(i, W1PC // P)
            if j == 0 and piece == 0:
                if sub < 2:
                    return w1_0a[:, k, P * sub:P * (sub + 1)]
                return w1p[(0, "0b")][:, k, P * (sub - 2):P * (sub - 1)]
            return w1p[(j, piece)][:, k, P * sub:P * (sub + 1)]

        csls = {}
        for j in range(S):
            csls[j] = []
            c0 = 0
            for cn in chunk_sizes[j]:
                csls[j].append((slice(c0, c0 + cn), cn))
                c0 += cn

        yt_sb = {}

        def gemm1(j):
            # yT[i] = silu(gate_i) * up_i, [P, c] per pair i. Quad structure:
            # 4 gate pairs then their 4 ups so the PE has gate work while the
            # up piece streams.
            yt_sb[j] = [None] * NPAIR
            for q in range(NPAIR // 4):
                quad = range(4 * q, 4 * q + 4)
                sgs = {}
                for i in quad:
                    yt_sb[j][i] = ypool.tile([P, cs[j]], DT, tag=f"yt{j}_{i}",
                                             name=f"yt{j}_{i}")
                for ci, (csl, cn) in enumerate(csls[j]):
                    for i in quad:
                        ps_g = pgpool.tile([P, cn], f32, tag="psg")
                        for k in range(KH):
                            nc.tensor.matmul(ps_g[:], lhsT=w1_slice(j, k, i),
                                             rhs=xt_sb[j][k][:, csl],
                                             start=(k == 0), stop=(k == KH - 1))
                        sg = apool.tile([P, cn], f32, tag=f"sg{i % 4}_{ci}")
                        nc.scalar.activation(sg[:], ps_g[:],
                                             mybir.ActivationFunctionType.Silu)
                        sgs[(i, ci)] = sg
                for ci, (csl, cn) in enumerate(csls[j]):
                    for i in quad:
                        ps_u = pupool.tile([P, cn], f32, tag="psu")
                        for k in range(KH):
                            nc.tensor.matmul(ps_u[:],
                                             lhsT=w1_slice(j, k, i + NPAIR),
                                             rhs=xt_sb[j][k][:, csl],
                                             start=(k == 0), stop=(k == KH - 1))
                        nc.vector.tensor_mul(yt_sb[j][i][:, csl],
                                             sgs[(i, ci)][:], ps_u[:])

        def gemm2(j, last=False):
            # chunk-outer / h-inner so each chunk's 8 h-tiles land in one
            # [P, NH, cn] slice of o_big and ship as ONE DMA — few out DMAs
            # keep the serialized HWDGE prep off the critical path. The very
            # last chunk instead ships per-h so the final transfers overlap
            # the remaining h-groups.
            o_big = opool.tile([P, NH, cs[j]], DT, tag=f"o{j}", name=f"o{j}")
            ov = out_d[j].rearrange("(h p) c -> p h c", p=P)
            n_chunks = len(csls[j])
            for ci, (csl, cn) in enumerate(csls[j]):
                tail = last and ci == n_chunks - 1
                for jh in range(NH):
                    ps_o = popool.tile([P, cn], f32, tag="pso")
                    for k in range(KI):
                        nc.tensor.matmul(
                            ps_o[:], lhsT=w2_sb[j][k][:, P * jh:P * (jh + 1)],
                            rhs=yt_sb[j][k][:, csl],
                            start=(k == 0), stop=(k == KI - 1))
                    if tail:
                        o_h = opool.tile([P, 1, cn], DT, tag=f"oh{jh}",
                                         name=f"oh{j}_{jh}")
                        nc.vector.tensor_mul(o_h[:, 0, :],
                                             sc_sb[j][:, csl], ps_o[:])
                        nc.sync.dma_start(out=ov[:, jh:jh + 1, csl],
                                          in_=o_h[:])
                    else:
                        nc.vector.tensor_mul(o_big[:, jh, csl],
                                             sc_sb[j][:, csl], ps_o[:])
                if not tail:
                    nc.sync.dma_start(out=ov[:, :, csl], in_=o_big[:, :, csl])

        for j in range(S):
            gemm1(j)
        # emit last the job whose final chunk is smallest: the tail
        # (mult + out DMA + drain) scales with the final chunk width
        order = sorted(range(S), key=lambda j: chunk_sizes[j][-1],
                       reverse=True)
        for j in order[:-1]:
            gemm2(j)
        gemm2(order[-1], last=True)

    nc.compile()
    return nc


def _make_in_maps(x, w1, w2, router_w):
    """Route + build per-core input shards. Returns (in_maps, meta) where
    meta = (idxs, jobs_per_core, cs) for unsharding."""
    idxs, wts = _route(x, router_w)
    loads = [len(i) for i in idxs]
    jobs_per_core, cs = _assign_jobs(loads)

    xt_f32 = x.T  # [H, T]
    cache = {}

    def job_tensors(e, q, c_pad):
        n = len(idxs[e])
        if (e, c_pad) not in cache:
            xt = np.zeros((H, c_pad), dtype=NP_DT)
            xt[:, :n] = xt_f32[:, idxs[e]].astype(NP_DT)
            sc = np.zeros((P, c_pad), dtype=np.float32)
            sc[:, :n] = wts[e][None, :]
            cache[(e, c_pad)] = (xt, sc)
        xt, sc = cache[(e, c_pad)]
        # gate rows [q*IH,(q+1)*IH) and up rows [INTER+q*IH, INTER+(q+1)*IH)
        w1j = np.concatenate([w1[e][q * IH:(q + 1) * IH],
                              w1[e][INTER + q * IH:INTER + (q + 1) * IH]],
                             axis=0)
        w2j = w2[e][:, q * IH:(q + 1) * IH]
        return {
            "xt": xt,
            "w1t": np.ascontiguousarray(w1j.T).astype(NP_DT),
            "w2t": np.ascontiguousarray(w2j.T).astype(NP_DT),
            "scale": sc,
        }

    in_maps = []
    for core in range(N_CORES):
        m = {}
        for s in range(S):
            e, q = jobs_per_core[core][s]
            t = job_tensors(e, q, cs[s])
            m[f"xt{s}"] = t["xt"]
            m[f"w1t{s}"] = t["w1t"]
            m[f"w2t{s}"] = t["w2t"]
            m[f"scale{s}"] = t["scale"]
        in_maps.append(m)
    return in_maps, (idxs, jobs_per_core, cs)


def kernel(hidden_states, w1, w2, router_w):
    x = np.ascontiguousarray(np.asarray(hidden_states, dtype=np.float32)
                             .reshape(T, H))
    w1 = np.asarray(w1, dtype=np.float32)
    w2 = np.asarray(w2, dtype=np.float32)
    router_w = np.asarray(router_w, dtype=np.float32)

    in_maps, (idxs, jobs_per_core, cs) = _make_in_maps(x, w1, w2, router_w)

    nc = _PROGRAM_CACHE.get(cs)
    if nc is None:
        nc = _PROGRAM_CACHE[cs] = _build_program(cs)

    try:
        res = run_bass_kernel_spmd(nc, in_maps, list(range(N_CORES)))
    except Exception:
        # transient runtime hiccups (e.g. mesh desync on a fresh session)
        # usually clear on retry
        res = run_bass_kernel_spmd(nc, in_maps, list(range(N_CORES)))

    out = np.zeros((T, H), dtype=np.float32)
    for core in range(N_CORES):
        for s in range(S):
            e, _q = jobs_per_core[core][s]
            n = len(idxs[e])
            if n:
                out[idxs[e]] += res.results[core][f"out{s}"][:, :n].T
    return out.reshape(1, T, H)


# revision 30
# speedup vs baseline: 1.1135x; 1.0561x over previous
"""MoE (top-2 of 8 experts, SwiGLU) on 8 Trainium2 NeuronCores.

Strategy (expert-parallel + quarter-expert load balancing):
  - Host computes the router and the top-2 dispatch (exact fp32 replica of
    the reference), yielding per-expert token lists + combine weights.
  - Each expert's MLP is split into TWO half-inter jobs (inter rows
    [h*1024,(h+1)*1024)): a job runs GEMM1 for its half of the gate/up
    rows and GEMM2 contracted over its half of INTER, producing a partial
    output for all of its expert's tokens. The halves are exact partial
    sums, added on the host. (A quarter-split S=4 was tried and rejected:
    its 4-deep GEMM2 accumulation groups are too short to hide the
    combine-mult latency in the 2-buffer PSUM ring, costing more than the
    extra balance saves.)
  - The 16 jobs are packed 2-per-core: slot s takes the jobs ranked
    [8s, 8s+8) by token count, padded to the slot max (538+508 columns for
    the key(0) routing vs 2*538 unbalanced) — the PE-bound cost scales
    with padded columns.
  - Per job: hT = w1q[j] @ x_jT (gate/up fused), yT = silu(g)*u,
    oT = (w2q[j] @ yT) * combine. All GEMMs bf16 with fp32 PSUM; outputs
    ship bf16 (host adds partials in fp32). fp8 was evaluated and rejected:
    plain e4m3 fails the 2e-2 gate (6.5e-2), and hi+lo residual
    compensation needs 3x the products, which loses at the ~1.9x measured
    DoubleRow speedup.
  - Emission: all GEMM1s first (job j+1's GEMM1 hides job j's yt latency,
    and w2 DMAs queue after all w1), then GEMM2s with the job owning the
    smallest final chunk last — the exposed tail (combine-mult + out DMA +
    drain) scales with the final chunk width.
  - The host packs job 0's x chunk-1 and the first 256 w1 gate columns
    into one "lead0" tensor in SBUF layout, so the first matmul is gated
    by a single leading DMA hop instead of two serialized prep+semaphore
    chains.

Layouts keep tokens on the PSUM free dim everywhere so no on-device
transposes are needed; weights are pre-transposed on the host.
"""

import sys

sys.path.insert(0, "/opt/trn_rl_repo")

import numpy as np
import ml_dtypes

import concourse.bass as bass  # noqa: F401  (bass must import before tile)
import concourse.tile as tile
from concourse import bacc, mybir
from concourse.bass_utils import run_bass_kernel_spmd

T = 2048
H = 1024
INTER = 2048
S = 2                  # inter-dim splits per expert (jobs per core)
IH = INTER // S        # 1024 inter rows per job
E = 8
TOPK = 2
N_CORES = 8
P = 128

DT = mybir.dt.bfloat16
NP_DT = ml_dtypes.bfloat16

_PROGRAM_CACHE = {}    # cs tuple -> compiled Bacc program

KH = H // P            # 8  k-tiles for GEMM1 (contract over H)
KI = IH // P           # 8  k-tiles for GEMM2 (contract over half INTER)
NPAIR = IH // P        # 8  gate/up pairs per job
NH = H // P            # 8  output h-tiles


def _route(x, router_w):
    """Replicates the reference router in fp32 numpy.

    Returns per-expert (token_indices, combine_weights)."""
    gating = (x @ router_w.T).astype(np.float32)              # [T, E]
    m = gating.max(axis=1, keepdims=True)
    p = np.exp(gating - m, dtype=np.float32)
    probs = p / p.sum(axis=1, keepdims=True)
    order = np.argsort(-probs, axis=1, kind="stable")         # ties -> lower idx
    sel = order[:, :TOPK]                                     # [T, K]
    topw = np.take_along_axis(probs, sel, axis=1)             # [T, K]

    idxs, wts = [], []
    for e in range(E):
        m_e = sel == e                                        # [T, K]
        rows = np.nonzero(m_e.any(axis=1))[0]
        idxs.append(rows.astype(np.int64))
        wts.append(topw[m_e].astype(np.float32))              # aligned with rows
    return idxs, wts


def _assign_jobs(loads):
    """S*E quarter-expert jobs -> N_CORES cores x S slots.

    Slot s holds the 8 jobs ranked [8s, 8s+8) by token count, padded to the
    slot max. Returns (jobs_per_core, cs): jobs_per_core[core][s] = (e, q),
    cs[s] = padded column count of slot s."""
    jobs = sorted(((loads[e], e, q) for e in range(E) for q in range(S)),
                  reverse=True)
    jobs_per_core = [[None] * S for _ in range(N_CORES)]
    cs = []
    for s in range(S):
        block = jobs[s * N_CORES:(s + 1) * N_CORES]
        cs.append(max(64, -(-block[0][0] // 2) * 2))
        for core, (_, e, q) in enumerate(block):
            jobs_per_core[core][s] = (e, q)
    return jobs_per_core, tuple(cs)


def _chunks(c):
    """Split c tokens into near-equal chunks of <=512 (PSUM bank limit).

    Chunks are kept >=256 where possible: below that LDWEIGHTS (~107 ns)
    stops hiding under the matmul stream and the PE goes weight-load-bound."""
    n = -(-c // 512)
    base = -(-(-(-c // n)) // 4) * 4                          # ceil(c/n) to mult of 4
    sizes = []
    left = c
    for _ in range(n - 1):
        sizes.append(base)
        left -= base
    sizes.append(left)
    return [s for s in sizes if s > 0]


def _build_program(cs, loop_n=0):
    """One SPMD program: S quarter-expert jobs (cs[s] padded tokens each).

    loop_n > 0 wraps the body in an on-device For_i loop (used only by the
    perf harness to measure the per-iteration slope)."""
    nc = bacc.Bacc("TRN2", target_bir_lowering=False, debug=False,
                   num_devices=N_CORES)
    f32 = mybir.dt.float32
    c1_0 = _chunks(cs[0])[0]
    # lead0 packs [xt0 chunk-1 | first 256 w1_0 gate cols] in SBUF layout
    # [p, k, col]: ONE leading DMA hop gates the first matmul instead of two
    # serialized prep+semaphore chains
    lead_d = nc.dram_tensor("lead0", [P, KH, c1_0 + 2 * P], DT,
                            kind="ExternalInput").ap()
    xt_d, w1t_d, w2t_d, sc_d, out_d = {}, {}, {}, {}, {}
    for j in range(S):
        xt_d[j] = nc.dram_tensor(f"xt{j}", [H, cs[j]], DT,
                                 kind="ExternalInput").ap()
        w1t_d[j] = nc.dram_tensor(f"w1t{j}", [H, 2 * IH], DT,
                                  kind="ExternalInput").ap()
        w2t_d[j] = nc.dram_tensor(f"w2t{j}", [IH, H], DT,
                                  kind="ExternalInput").ap()
        sc_d[j] = nc.dram_tensor(f"scale{j}", [P, cs[j]], f32,
                                 kind="ExternalInput").ap()
        out_d[j] = nc.dram_tensor(f"out{j}", [H, cs[j]], DT,
                                  kind="ExternalOutput").ap()

    from contextlib import ExitStack
    with tile.TileContext(nc) as tc, ExitStack() as ctx:
        wpool = ctx.enter_context(tc.tile_pool(name="weights", bufs=1))
        xpool = ctx.enter_context(tc.tile_pool(name="xt", bufs=1))
        ypool = ctx.enter_context(tc.tile_pool(name="yt", bufs=1))
        apool = ctx.enter_context(tc.tile_pool(name="act", bufs=2))
        opool = ctx.enter_context(tc.tile_pool(name="ot", bufs=1))
        pgpool = ctx.enter_context(tc.tile_pool(name="psg", bufs=3, space="PSUM"))
        pupool = ctx.enter_context(tc.tile_pool(name="psu", bufs=3, space="PSUM"))
        popool = ctx.enter_context(tc.tile_pool(name="pso", bufs=2, space="PSUM"))

        if loop_n:
            loop = ctx.enter_context(tc.For_i(
                0, loop_n, 1,
                hint_engines=(mybir.EngineType.PE, mybir.EngineType.SP,
                              mybir.EngineType.Activation, mybir.EngineType.DVE)))

        # ---- PE warmup ----
        # Dependency-free matmuls on an (uninitialized) scratch tile warm the
        # PE HAM clock-gate to 2.4 GHz during the initial DMA wait.
        warm_sb = xpool.tile([P, P], DT, tag="warm")
        nc.vector.memset(warm_sb[:, 0:1], 0.0)
        ps_w = popool.tile([P, P], f32, tag="pso", name="ps_warm")
        for _ in range(50):
            nc.tensor.matmul(ps_w[:], lhsT=warm_sb[:], rhs=warm_sb[:],
                             start=True, stop=True)

        # ---- input loads ----
        # One merged DMA per logical tensor/piece: HWDGE prep (~625 ns) is
        # per-instruction and serialized with transfers on the single queue,
        # so fewer+bigger DMAs win; the leading pieces gate the first MMs.
        xt_t, xt_sb, sc_sb, w1p = {}, {}, {}, {}
        chunk_sizes = {j: _chunks(cs[j]) for j in range(S)}

        W1PC = 512  # w1 piece: 512 cols (4 pairs' gate or up halves)

        def load_w1_cols(j, lo, hi, tag):
            t = wpool.tile([P, KH, hi - lo], DT, tag=tag, name=tag)
            nc.sync.dma_start(
                out=t[:], in_=w1t_d[j][:, lo:hi].rearrange("(k p) c -> p k c", p=P))
            return t

        # job 0 leads with the packed lead0 tensor (x chunk-1 + first 256
        # gate cols in one DMA), then rest of gate, x rest — in PE
        # consumption order.
        c1 = c1_0
        lead_t = xpool.tile([P, KH, c1 + 2 * P], DT, tag="lead0",
                            name="lead0")
        nc.sync.dma_start(out=lead_t[:], in_=lead_d[:])
        w1p[(0, "0b")] = load_w1_cols(0, 2 * P, W1PC, "w1_0_0b")
        if c1 < cs[0]:
            xt_t[0] = xpool.tile([P, KH, cs[0] - c1], DT, tag="xt0",
                                 name="xt0")
            xv0 = xt_d[0].rearrange("(k p) c -> p k c", p=P)
            nc.sync.dma_start(out=xt_t[0][:], in_=xv0[:, :, c1:])
        # remaining job-0 pieces: gate piece p feeds pairs 4p..4p+3 paired
        # with up piece p+2
        for piece in (2, 1, 3):
            w1p[(0, piece)] = load_w1_cols(0, piece * W1PC, (piece + 1) * W1PC,
                                           f"w1_0_{piece}")
        # jobs 1..S-1: x then w1 pieces in consumption order
        for j in range(1, S):
            xt_t[j] = xpool.tile([P, KH, cs[j]], DT, tag=f"xt{j}",
                                 name=f"xt{j}")
            nc.sync.dma_start(out=xt_t[j][:],
                              in_=xt_d[j].rearrange("(k p) c -> p k c", p=P))
            for piece in (0, 2, 1, 3):
                w1p[(j, piece)] = load_w1_cols(j, piece * W1PC,
                                               (piece + 1) * W1PC,
                                               f"w1_{j}_{piece}")

        for j in range(1, S):
            xt_sb[j] = [xt_t[j][:, k, :] for k in range(KH)]

        def xsrc(j, k, ci, csl, cn):
            # rhs source for job j, k-tile k, chunk ci (csl is the global
            # column slice): job 0 chunk 1 lives in lead_t, its later
            # chunks in the (chunk-1-offset) xt0 tile
            if j == 0:
                if ci == 0:
                    return lead_t[:, k, :cn]
                return xt_t[0][:, k, csl.start - c1:csl.stop - c1]
            return xt_sb[j][k][:, csl]

        # w2: one merged DMA per job, after all w1
        w2_sb = {}
        for j in range(S):
            t = wpool.tile([P, KI, H], DT, tag=f"w2_{j}", name=f"w2_{j}")
            nc.sync.dma_start(
                out=t[:], in_=w2t_d[j].rearrange("(k p) c -> p k c", p=P))
            w2_sb[j] = [t[:, k, :] for k in range(KI)]

        for j in range(S):
            sc_sb[j] = xpool.tile([P, cs[j]], f32, tag=f"sc{j}", name=f"sc{j}")
            nc.sync.dma_start(out=sc_sb[j][:], in_=sc_d[j][:])

        def w1_slice(j, k, i):
            # stationary lhsT [P(h), P(inter)] for job-local inter tile i
            # (0..2*NPAIR-1: NPAIR gate then NPAIR up)
            piece, sub = divmod(i, W1PC // P)
            if j == 0 and piece == 0:
                if sub < 2:
                    return lead_t[:, k, c1 + P * sub:c1 + P * (sub + 1)]
                return w1p[(0, "0b")][:, k, P * (sub - 2):P * (sub - 1)]
            return w1p[(j, piece)][:, k, P * sub:P * (sub + 1)]

        csls = {}
        for j in range(S):
            csls[j] = []
            c0 = 0
            for cn in chunk_sizes[j]:
                csls[j].append((slice(c0, c0 + cn), cn))
                c0 += cn

        yt_sb = {}

        def gemm1(j):
            # yT[i] = silu(gate_i) * up_i, [P, c] per pair i. Quad structure:
            # 4 gate pairs then their 4 ups so the PE has gate work while the
            # up piece streams.
            yt_sb[j] = [None] * NPAIR
            for q in range(NPAIR // 4):
                quad = range(4 * q, 4 * q + 4)
                sgs = {}
                for i in quad:
                    yt_sb[j][i] = ypool.tile([P, cs[j]], DT, tag=f"yt{j}_{i}",
                                             name=f"yt{j}_{i}")
                for ci, (csl, cn) in enumerate(csls[j]):
                    for i in quad:
                        ps_g = pgpool.tile([P, cn], f32, tag="psg")
                        for k in range(KH):
                            nc.tensor.matmul(ps_g[:], lhsT=w1_slice(j, k, i),
                                             rhs=xsrc(j, k, ci, csl, cn),
                                             start=(k == 0), stop=(k == KH - 1))
                        sg = apool.tile([P, cn], f32, tag=f"sg{i % 4}_{ci}")
                        nc.scalar.activation(sg[:], ps_g[:],
                                             mybir.ActivationFunctionType.Silu)
                        sgs[(i, ci)] = sg
                for ci, (csl, cn) in enumerate(csls[j]):
                    for i in quad:
                        ps_u = pupool.tile([P, cn], f32, tag="psu")
                        for k in range(KH):
                            nc.tensor.matmul(ps_u[:],
                                             lhsT=w1_slice(j, k, i + NPAIR),
                                             rhs=xsrc(j, k, ci, csl, cn),
                                             start=(k == 0), stop=(k == KH - 1))
                        nc.vector.tensor_mul(yt_sb[j][i][:, csl],
                                             sgs[(i, ci)][:], ps_u[:])

        def gemm2(j, last=False):
            # chunk-outer / h-inner so each chunk's 8 h-tiles land in one
            # [P, NH, cn] slice of o_big and ship as ONE DMA — few out DMAs
            # keep the serialized HWDGE prep off the critical path. The very
            # last chunk instead ships per-h so the final transfers overlap
            # the remaining h-groups.
            o_big = opool.tile([P, NH, cs[j]], DT, tag=f"o{j}", name=f"o{j}")
            ov = out_d[j].rearrange("(h p) c -> p h c", p=P)
            n_chunks = len(csls[j])
            for ci, (csl, cn) in enumerate(csls[j]):
                tail = last and ci == n_chunks - 1
                for jh in range(NH):
                    ps_o = popool.tile([P, cn], f32, tag="pso")
                    for k in range(KI):
                        nc.tensor.matmul(
                            ps_o[:], lhsT=w2_sb[j][k][:, P * jh:P * (jh + 1)],
                            rhs=yt_sb[j][k][:, csl],
                            start=(k == 0), stop=(k == KI - 1))
                    if tail:
                        o_h = opool.tile([P, 1, cn], DT, tag=f"oh{jh}",
                                         name=f"oh{j}_{jh}")
                        nc.vector.tensor_mul(o_h[:, 0, :],
                                             sc_sb[j][:, csl], ps_o[:])
                        nc.sync.dma_start(out=ov[:, jh:jh + 1, csl],
                                          in_=o_h[:])
                    else:
                        nc.vector.tensor_mul(o_big[:, jh, csl],
                                             sc_sb[j][:, csl], ps_o[:])
                if not tail:
                    nc.sync.dma_start(out=ov[:, :, csl], in_=o_big[:, :, csl])

        for j in range(S):
            gemm1(j)
        # emit last the job whose final chunk is smallest: the tail
        # (mult + out DMA + drain) scales with the final chunk width
        order = sorted(range(S), key=lambda j: chunk_sizes[j][-1],
                       reverse=True)
        for j in order[:-1]:
            gemm2(j)
        gemm2(order[-1], last=True)

    nc.compile()
    return nc


def _make_in_maps(x, w1, w2, router_w):
    """Route + build per-core input shards. Returns (in_maps, meta) where
    meta = (idxs, jobs_per_core, cs) for unsharding."""
    idxs, wts = _route(x, router_w)
    loads = [len(i) for i in idxs]
    jobs_per_core, cs = _assign_jobs(loads)

    xt_f32 = x.T  # [H, T]
    cache = {}

    def job_tensors(e, q, c_pad):
        n = len(idxs[e])
        if (e, c_pad) not in cache:
            xt = np.zeros((H, c_pad), dtype=NP_DT)
            xt[:, :n] = xt_f32[:, idxs[e]].astype(NP_DT)
            sc = np.zeros((P, c_pad), dtype=np.float32)
            sc[:, :n] = wts[e][None, :]
            cache[(e, c_pad)] = (xt, sc)
        xt, sc = cache[(e, c_pad)]
        # gate rows [q*IH,(q+1)*IH) and up rows [INTER+q*IH, INTER+(q+1)*IH)
        w1j = np.concatenate([w1[e][q * IH:(q + 1) * IH],
                              w1[e][INTER + q * IH:INTER + (q + 1) * IH]],
                             axis=0)
        w2j = w2[e][:, q * IH:(q + 1) * IH]
        return {
            "xt": xt,
            "w1t": np.ascontiguousarray(w1j.T).astype(NP_DT),
            "w2t": np.ascontiguousarray(w2j.T).astype(NP_DT),
            "scale": sc,
        }

    c1_0 = _chunks(cs[0])[0]

    def rearr(a):
        # [H, c] -> [p, k, c] (the SBUF layout of "(k p) c -> p k c")
        return np.ascontiguousarray(
            a.reshape(KH, P, a.shape[1]).transpose(1, 0, 2))

    in_maps = []
    for core in range(N_CORES):
        m = {}
        for s in range(S):
            e, q = jobs_per_core[core][s]
            t = job_tensors(e, q, cs[s])
            m[f"xt{s}"] = t["xt"]
            m[f"w1t{s}"] = t["w1t"]
            m[f"w2t{s}"] = t["w2t"]
            m[f"scale{s}"] = t["scale"]
        lead = np.empty((P, KH, c1_0 + 2 * P), dtype=NP_DT)
        lead[:, :, :c1_0] = rearr(m["xt0"])[:, :, :c1_0]
        lead[:, :, c1_0:] = rearr(m["w1t0"][:, :2 * P])
        m["lead0"] = lead
        in_maps.append(m)
    return in_maps, (idxs, jobs_per_core, cs)


def kernel(hidden_states, w1, w2, router_w):
    x = np.ascontiguousarray(np.asarray(hidden_states, dtype=np.float32)
                             .reshape(T, H))
    w1 = np.asarray(w1, dtype=np.float32)
    w2 = np.asarray(w2, dtype=np.float32)
    router_w = np.asarray(router_w, dtype=np.float32)

    in_maps, (idxs, jobs_per_core, cs) = _make_in_maps(x, w1, w2, router_w)

    nc = _PROGRAM_CACHE.get(cs)
    if nc is None:
        nc = _PROGRAM_CACHE[cs] = _build_program(cs)

    try:
        res = run_bass_kernel_spmd(nc, in_maps, list(range(N_CORES)))
    except Exception:
        # transient runtime hiccups (e.g. mesh desync on a fresh session)
        # usually clear on retry
        res = run_bass_kernel_spmd(nc, in_maps, list(range(N_CORES)))

    out = np.zeros((T, H), dtype=np.float32)
    for core in range(N_CORES):
        for s in range(S):
            e, _q = jobs_per_core[core][s]
            n = len(idxs[e])
            if n:
                out[idxs[e]] += res.results[core][f"out{s}"][:, :n].T
    return out.reshape(1, T, H)
